# revision 1
# baseline (speedup 1.0000x reference)
"""Sparse single-head attention (QKV proj + key-padding mask + softmax) on 8 trn2 cores.

Math per batch element b (one NeuronCore each):
    qh = q @ Wq + bq ; kh = k @ Wk + bk ; vh = v @ Wv + bv        [S, 64]
    scores = qh @ kh^T / 8 ; scores[:, mask==0] = -1e10
    out = softmax(scores, -1) @ vh                                 [S, 64]

Device strategy:
  - Host gathers the unmasked k/v rows (mask is ~50% zeros) and pads to SK
    (multiple of 256); padded keys get an additive bias of -1e10 so their
    exp() underflows to exactly 0 - identical math to the reference.
  - All of q/k/v must be d-major on chip (PE contracts over partitions), so
    natural [128, 512] tiles are PE-transposed in 128x128 blocks (kept fp32:
    transposes must be lossless).
  - All projection / scores / output matmuls run with operands bitcast to
    float32r: full PE stream rate (1 cycle/row at N>=256) vs 4 cycles/row
    for plain fp32.
  - qh^T is augmented with a row of ones and kh^T with a row of mask biases:
    the scores matmul fuses the additive mask for free.  1/sqrt(64) is
    folded into Wq/bq on the host.
  - scores are computed TRANSPOSED ([k, q] layout): softmax exp is
    layout-agnostic, the sum over k comes free from a ones-column in vh
    (row 64 of the accumulator = sum of exps), and attn^T is exactly what
    the out-matmul needs as lhsT - no per-tile attention transposes.
  - exp() is not max-stabilized: scores ~ N(0, 0.11) for this input
    distribution, far inside fp32 exp range; masked lanes are -1e10 which
    underflows to +0 exactly like the stabilized reference.
  - v is projected in the same transposed layout (full-rate N=512 matmuls)
    then flipped back per 128-key chunk with cheap [65,128] PE transposes.
"""

import numpy as np

import concourse.bass as bass
import concourse.tile as tile
from concourse import bacc, mybir
from concourse.bass_utils import run_bass_kernel_spmd
from concourse.masks import make_identity

F32 = mybir.dt.float32
F32R = mybir.dt.float32r
S = 4096  # query rows per core
D = 512  # model dim
DK = 64  # head dim (q/k and v)
N_CORES = 8


def _r(ap):
    return ap.bitcast(F32R)


def _build_nc(SK: int):
    """Build the single-core Bass program (same program on all 8 cores)."""
    assert SK % 128 == 0
    SKC = SK // 128  # 128-row key chunks
    # group key chunks in pairs for the exp (one ACT op per pair); odd tail ok
    groups = []
    kc = 0
    while kc < SKC:
        g = min(2, SKC - kc)
        groups.append((kc, g))
        kc += g

    nc = bacc.Bacc("TRN2", target_bir_lowering=False, debug=False)

    q_d = nc.dram_tensor("q", [S, D], F32, kind="ExternalInput").ap()
    kg_d = nc.dram_tensor("kg", [SK, D], F32, kind="ExternalInput").ap()
    vg_d = nc.dram_tensor("vg", [SK, D], F32, kind="ExternalInput").ap()
    mb_d = nc.dram_tensor("mbias", [1, SK], F32, kind="ExternalInput").ap()
    wq_d = nc.dram_tensor("wq", [D, DK], F32, kind="ExternalInput").ap()
    wk_d = nc.dram_tensor("wk", [D, DK], F32, kind="ExternalInput").ap()
    wv_d = nc.dram_tensor("wv", [D, DK + 1], F32, kind="ExternalInput").ap()
    bq_d = nc.dram_tensor("bq", [DK, 1], F32, kind="ExternalInput").ap()
    bk_d = nc.dram_tensor("bk", [DK, 1], F32, kind="ExternalInput").ap()
    bv_d = nc.dram_tensor("bv", [DK + 1, 1], F32, kind="ExternalInput").ap()
    out_d = nc.dram_tensor("out", [S, DK], F32, kind="ExternalOutput").ap()

    with tile.TileContext(nc) as tc:
        with (
            tc.tile_pool(name="persist", bufs=1) as persist,
            tc.tile_pool(name="consts", bufs=1) as consts,
            tc.tile_pool(name="nat", bufs=3) as nat_pool,
            tc.tile_pool(name="xt", bufs=3) as xt_pool,
            tc.tile_pool(name="ps", bufs=2, space="PSUM") as pp,
            tc.tile_pool(name="expp", bufs=3) as exp_pool,
            tc.tile_pool(name="otp", bufs=2) as ot_pool,
            tc.tile_pool(name="recp", bufs=8) as rec_pool,
            tc.tile_pool(name="outp", bufs=2) as out_pool,
        ):
            # k block 0 leads the HWDGE queue; consts go via SWDGE (gpsimd)
            nat_k0 = nat_pool.tile([128, 4, D], F32, tag="nat")
            nc.sync.dma_start(
                nat_k0[:, :, :], kg_d[0:512, :].rearrange("(t p) d -> p t d", p=128)
            )

            ident = consts.tile([128, 128], F32)
            make_identity(nc, ident[:, :])

            wq = consts.tile([128, 4, DK], F32R)
            wk = consts.tile([128, 4, DK], F32R)
            wv = consts.tile([128, 4, DK + 1], F32R)
            wq_f = consts.tile([128, 4, DK], F32)
            wk_f = consts.tile([128, 4, DK], F32)
            wv_f = consts.tile([128, 4, DK + 1], F32)
            nc.gpsimd.dma_start(wq_f[:, :, :], wq_d.rearrange("(c p) k -> p c k", p=128))
            nc.gpsimd.dma_start(wk_f[:, :, :], wk_d.rearrange("(c p) k -> p c k", p=128))
            nc.gpsimd.dma_start(wv_f[:, :, :], wv_d.rearrange("(c p) k -> p c k", p=128))
            nc.vector.tensor_copy(wq[:, :, :], wq_f[:, :, :])
            nc.vector.tensor_copy(wk[:, :, :], wk_f[:, :, :])
            nc.vector.tensor_copy(wv[:, :, :], wv_f[:, :, :])
            bq = consts.tile([DK, 1], F32)
            bk = consts.tile([DK, 1], F32)
            bv = consts.tile([DK + 1, 1], F32)
            nc.gpsimd.dma_start(bq[:, :], bq_d)
            nc.gpsimd.dma_start(bk[:, :], bk_d)
            nc.gpsimd.dma_start(bv[:, :], bv_d)

            qhT = persist.tile([DK + 1, S], F32R)  # row 64 = ones
            khT = persist.tile([DK + 1, SK], F32R)  # row 64 = mask bias
            vhT = persist.tile([DK + 1, SK], F32)  # row 64 = ones
            vh = persist.tile([128, SKC, DK + 1], F32R)  # col 64 = ones
            ones_f = consts.tile([1, S], F32)
            nc.vector.memset(ones_f[:, :], 1.0)
            nc.vector.tensor_copy(qhT[DK : DK + 1, :], ones_f[:, :])
            mb_f = consts.tile([1, SK], F32)
            nc.gpsimd.dma_start(mb_f[:, :], mb_d)
            nc.vector.tensor_copy(khT[DK : DK + 1, :], mb_f[:, :])

            evac_ctr = [0]

            def load_and_transpose(src_ap, r0, nrows, preloaded=None, act_share=3):
                """DMA rows [r0, r0+nrows) and PE-transpose to d-major.

                Evacuates one [128, nrows] psum tile per d-chunk; every
                act_share-th evacuation goes to ACT (0 = all DVE).
                """
                nt = nrows // 128
                if preloaded is not None:
                    nat = preloaded
                else:
                    nat = nat_pool.tile([128, 4, D], F32, tag="nat")
                    nc.sync.dma_start(
                        nat[:, 0:nt, :],
                        src_ap[r0 : r0 + nrows, :].rearrange("(t p) d -> p t d", p=128),
                    )
                xt = xt_pool.tile([128, 4, 512], F32R, tag="xt")
                for c in range(4):
                    ps = pp.tile([128, 512], F32, tag="tr")
                    for t in range(nt):
                        nc.tensor.transpose(
                            ps[:, t * 128 : (t + 1) * 128],
                            nat[:, t, c * 128 : (c + 1) * 128],
                            ident[:, :],
                        )
                    dst = xt[:, c, 0:nrows]
                    use_act = act_share and evac_ctr[0] % act_share == act_share - 1
                    evac_ctr[0] += 1
                    if use_act:
                        nc.scalar.copy(dst, ps[:, 0:nrows])
                    else:
                        nc.vector.tensor_copy(dst, ps[:, 0:nrows])
                return xt

            def project(xt, w, dst, bias, c0, ncols):
                """dst[:, c0:c0+ncols] = w.T @ x^T + bias (per-partition)."""
                m = w.shape[2]
                ps = pp.tile([DK + 1, 512], F32, tag="opr")
                for c in range(4):
                    nc.tensor.matmul(
                        ps[0:m, 0:ncols],
                        w[:, c, :],
                        xt[:, c, 0:ncols],
                        start=(c == 0),
                        stop=(c == 3),
                    )
                nc.vector.tensor_scalar_add(
                    dst[0:m, c0 : c0 + ncols], ps[0:m, 0:ncols], bias[0:m, :]
                )

            # ---- Phase A: K and V paths (pipelined), then vh flips ----
            kv_blocks = []
            r0 = 0
            while r0 < SK:
                nrows = min(512, SK - r0)
                kv_blocks.append((r0, nrows))
                r0 += nrows
            work = [("k", r0, nr) for r0, nr in kv_blocks] + [
                ("v", r0, nr) for r0, nr in kv_blocks
            ]
            pending = None
            for i, (kind, r0, nr) in enumerate(work):
                src = kg_d if kind == "k" else vg_d
                pre = nat_k0 if i == 0 else None
                xt = load_and_transpose(src, r0, nr, preloaded=pre, act_share=2)
                if pending is not None:
                    pk, pr0, pnr, pxt = pending
                    project(pxt, wk if pk == "k" else wv, khT if pk == "k" else vhT,
                            bk if pk == "k" else bv, pr0, pnr)
                pending = (kind, r0, nr, xt)
            pk, pr0, pnr, pxt = pending
            project(pxt, wv, vhT, bv, pr0, pnr)

            # flip vhT -> vh, 4 chunks per psum tile, one evacuation each
            for kc0 in range(0, SKC, 4):
                n = min(4, SKC - kc0)
                ps = pp.tile([128, 4, 128], F32, tag="tr")
                for i in range(n):
                    kc = kc0 + i
                    nc.tensor.transpose(
                        ps[:, i, 0 : DK + 1],
                        vhT[:, kc * 128 : (kc + 1) * 128],
                        ident[0 : DK + 1, 0 : DK + 1],
                    )
                nc.vector.tensor_copy(
                    vh[:, kc0 : kc0 + n, :], ps[:, 0:n, 0 : DK + 1]
                )

            # ---- Phase B: merged q-projection + attention, one block ahead ----
            def prep(qb):
                xt = load_and_transpose(q_d, qb * 512, 512, act_share=0)
                project(xt, wq, qhT, bq, qb * 512, 512)

            prep(0)
            for qb in range(S // 512):
                if qb + 1 < S // 512:
                    prep(qb + 1)
                qs = qhT[:, qb * 512 : (qb + 1) * 512]
                po = pp.tile([DK + 1, 512], F32, tag="opr")
                prev = None
                for kc0, g in groups:
                    pscore = pp.tile([128, 1024], F32, tag="s")
                    for h in range(g):
                        nc.tensor.matmul(
                            pscore[:, h * 512 : (h + 1) * 512],
                            khT[:, (kc0 + h) * 128 : (kc0 + h + 1) * 128],
                            qs,
                            start=True,
                            stop=True,
                        )
                    et = exp_pool.tile([128, 1024], F32R, tag="e")
                    nc.scalar.activation(
                        et[:, 0 : g * 512],
                        pscore[:, 0 : g * 512],
                        mybir.ActivationFunctionType.Exp,
                    )
                    if prev is not None:
                        pet, pkc0, pg = prev
                        for h in range(pg):
                            kc = pkc0 + h
                            nc.tensor.matmul(
                                po[:, :],
                                vh[:, kc, :],
                                pet[:, h * 512 : (h + 1) * 512],
                                start=(kc == 0),
                                stop=False,
                            )
                    prev = (et, kc0, g)
                pet, pkc0, pg = prev
                for h in range(pg):
                    kc = pkc0 + h
                    nc.tensor.matmul(
                        po[:, :],
                        vh[:, kc, :],
                        pet[:, h * 512 : (h + 1) * 512],
                        start=(kc == 0),
                        stop=(h == pg - 1),
                    )

                # finalize: transpose back (4 packed per psum tile), 1/sum scale
                ot = ot_pool.tile([DK + 1, 512], F32, tag="ot")
                nc.vector.tensor_copy(ot[:, :], po[:, :])
                ostage = out_pool.tile([128, 4, DK], F32, tag="os")
                pf = pp.tile([128, 4, 128], F32, tag="tr")
                for t in range(4):
                    nc.tensor.transpose(
                        pf[:, t, 0 : DK + 1],
                        ot[:, t * 128 : (t + 1) * 128],
                        ident[0 : DK + 1, 0 : DK + 1],
                    )
                for t in range(4):
                    rec = rec_pool.tile([128, 1], F32, tag="r")
                    nc.vector.reciprocal(rec[:, :], pf[:, t, DK : DK + 1])
                    nc.vector.tensor_scalar_mul(
                        ostage[:, t, :], pf[:, t, 0:DK], rec[:, :]
                    )
                nc.sync.dma_start(
                    out_d[qb * 512 : (qb + 1) * 512, :].rearrange(
                        "(t p) v -> p t v", p=128
                    ),
                    ostage[:, :, :],
                )

    nc.compile()
    return nc


_NC_CACHE: dict = {}


def prepare(inputs):
    """Host-side preprocessing: returns (nc, in_maps)."""
    q = np.ascontiguousarray(inputs["q"], dtype=np.float32)
    k = np.ascontiguousarray(inputs["k"], dtype=np.float32)
    v = np.ascontiguousarray(inputs["v"], dtype=np.float32)
    mask = np.asarray(inputs["mask"])
    Wq = np.asarray(inputs["Wq"], dtype=np.float32)
    bq = np.asarray(inputs["bq"], dtype=np.float32)
    Wk = np.asarray(inputs["Wk"], dtype=np.float32)
    bk = np.asarray(inputs["bk"], dtype=np.float32)
    Wv = np.asarray(inputs["Wv"], dtype=np.float32)
    bv = np.asarray(inputs["bv"], dtype=np.float32)
    B = q.shape[0]
    assert q.shape == (B, S, D) and B == N_CORES

    # gather unmasked key/value rows per batch; pad to a common SK
    idxs = [np.nonzero(mask[b])[0] for b in range(B)]
    max_cnt = max(len(ix) for ix in idxs)
    SK = ((max_cnt + 127) // 128) * 128
    SK = max(SK, 512)

    scale = 1.0 / np.sqrt(np.float32(DK))
    Wq8 = (Wq * scale).astype(np.float32)
    bq8 = (bq * scale).astype(np.float32).reshape(DK, 1)
    bk2 = bk.astype(np.float32).reshape(DK, 1)
    Wv_aug = np.concatenate([Wv, np.zeros((D, 1), np.float32)], axis=1)
    bv_aug = np.concatenate([bv, np.ones(1, np.float32)]).reshape(DK + 1, 1)

    in_maps = []
    for b in range(B):
        ix = idxs[b]
        cnt = len(ix)
        kg = np.zeros((SK, D), np.float32)
        vg = np.zeros((SK, D), np.float32)
        kg[:cnt] = k[b][ix]
        vg[:cnt] = v[b][ix]
        mb = np.zeros((1, SK), np.float32)
        mb[0, cnt:] = -1e10
        in_maps.append(
            dict(
                q=q[b],
                kg=kg,
                vg=vg,
                mbias=mb,
                wq=Wq8,
                wk=Wk.astype(np.float32),
                wv=Wv_aug,
                bq=bq8,
                bk=bk2,
                bv=bv_aug,
            )
        )

    if SK not in _NC_CACHE:
        _NC_CACHE[SK] = _build_nc(SK)
    return _NC_CACHE[SK], in_maps


def kernel(**inputs) -> np.ndarray:
    nc, in_maps = prepare(inputs)
    res = run_bass_kernel_spmd(nc, in_maps, list(range(N_CORES)))
    out = np.stack([res.results[b]["out"] for b in range(len(in_maps))], axis=0)
    return out.astype(np.float32)



# revision 5
# speedup vs baseline: 1.2338x; 1.2338x over previous
"""Sparse single-head attention (QKV proj + key-padding mask + softmax) on 8 trn2 cores.

Math per batch element b (one NeuronCore each):
    qh = q @ Wq + bq ; kh = k @ Wk + bk ; vh = v @ Wv + bv        [S, 64]
    scores = qh @ kh^T / 8 ; scores[:, mask==0] = -1e10
    out = softmax(scores, -1) @ vh                                 [S, 64]

Device strategy (v2):
  - Host gathers the unmasked k/v rows (~50% of keys), casts q/k/v to bf16
    and stores them d-major (pre-transposed), so the kernel needs NO input
    transposes on the PE.  1/sqrt(64) is folded into Wq.
  - All matmuls run in bf16 (1 cycle/row at any N); accumulation stays fp32
    in PSUM, so precision loss is only the 0.4% bf16 input rounding.
  - Padded keys are neutralized without any mask bias: v is projected with an
    augmented ones-row (vg_aug = [v, 1], Wv_aug row 512 = [bv, 1]) so padded
    rows produce vh == 0 (including the ones-column used for the softmax
    denominator).  exp(score_pad) is finite garbage multiplied by zero.
  - scores are computed TRANSPOSED ([k, q] layout) with K=64 contraction,
    packed TWO key-chunks per PE pass via row tiling: chunk pairs live on
    partition halves (khT2 [128, pair, 128]) and qhT is duplicated onto both
    halves by a col-tiled projection; the two 64-row matmuls run concurrently
    in disjoint row groups of the array -> ~2x score throughput.
  - exp() is not max-stabilized: scores ~ N(0, 0.11), far inside range; the
    sum over keys comes free from the ones-column of vh (row 64 of the AV
    accumulator).
  - attn^T @ vh runs with vh natural ([key, 65]) as the stationary operand,
    K=128 full-array contraction, N=512 moving.
"""

import numpy as np
import ml_dtypes

import concourse.bass as bass
import concourse.tile as tile
from concourse import bacc, mybir
from concourse.bass_utils import run_bass_kernel_spmd
from concourse.masks import make_identity

F32 = mybir.dt.float32
BF16 = mybir.dt.bfloat16
NP_BF16 = ml_dtypes.bfloat16
S = 4096  # query rows per core
D = 512  # model dim
DK = 64  # head dim (q/k and v)
N_CORES = 8
EXP = mybir.ActivationFunctionType.Exp


def _build_nc(SK: int):
    """Build the single-core Bass program (same program on all 8 cores)."""
    assert SK % 128 == 0
    SKC = SK // 128  # 128-key chunks
    NPAIR = (SKC + 1) // 2  # chunk pairs (last may be a lone even half)
    QB = S // 512  # 512-query blocks

    nc = bacc.Bacc("TRN2", target_bir_lowering=False, debug=False)

    qt_d = nc.dram_tensor("qt", [D, S], BF16, kind="ExternalInput").ap()
    kt_d = nc.dram_tensor("kt", [D, SK], BF16, kind="ExternalInput").ap()
    vt_d = nc.dram_tensor("vt", [D, SK], BF16, kind="ExternalInput").ap()
    vones_d = nc.dram_tensor("vones", [1, SK], BF16, kind="ExternalInput").ap()
    wq_d = nc.dram_tensor("wq", [D, DK], BF16, kind="ExternalInput").ap()
    wk_d = nc.dram_tensor("wk", [D, DK], BF16, kind="ExternalInput").ap()
    wv_d = nc.dram_tensor("wv", [D, DK + 1], BF16, kind="ExternalInput").ap()
    wvl_d = nc.dram_tensor("wvl", [1, DK + 1], BF16, kind="ExternalInput").ap()
    bq_d = nc.dram_tensor("bq2", [128, 1], F32, kind="ExternalInput").ap()
    bk_d = nc.dram_tensor("bk2", [128, 1], F32, kind="ExternalInput").ap()
    out_d = nc.dram_tensor("out", [S, DK], F32, kind="ExternalOutput").ap()

    kv_blocks = []
    r0 = 0
    while r0 < SK:
        nr = min(512, SK - r0)
        kv_blocks.append((r0, nr))
        r0 += nr

    with tile.TileContext(nc) as tc:
        with (
            tc.tile_pool(name="consts", bufs=1) as consts,
            tc.tile_pool(name="persist", bufs=1) as persist,
            tc.tile_pool(name="kvp", bufs=len(kv_blocks)) as kv_pool,
            tc.tile_pool(name="qtp", bufs=3) as qt_pool,
            tc.tile_pool(name="qhp", bufs=2) as qh_pool,
            tc.tile_pool(name="etp", bufs=3) as et_pool,
            tc.tile_pool(name="stg", bufs=2) as stg_pool,
            tc.tile_pool(name="outp", bufs=2) as out_pool,
            tc.tile_pool(name="recp", bufs=4) as rec_pool,
            tc.tile_pool(name="ppa", bufs=2, space="PSUM") as ppa,
            tc.tile_pool(name="pps", bufs=2, space="PSUM") as pps,
            tc.tile_pool(name="ppo", bufs=2, space="PSUM") as ppo,
        ):
            # ---- input DMAs lead the HWDGE queue: k blocks, v blocks, q ----
            kt_tiles = []
            for r0, nr in kv_blocks:
                t = kv_pool.tile([128, 4, 512], BF16, tag="kt")
                nc.sync.dma_start(
                    t[:, :, 0:nr],
                    kt_d.rearrange("(c p) s -> p c s", p=128)[:, :, r0 : r0 + nr],
                )
                kt_tiles.append(t)
            vt_tiles = []
            for r0, nr in kv_blocks:
                t = kv_pool.tile([128, 4, 512], BF16, tag="vt")
                nc.sync.dma_start(
                    t[:, :, 0:nr],
                    vt_d.rearrange("(c p) s -> p c s", p=128)[:, :, r0 : r0 + nr],
                )
                vt_tiles.append(t)

            qt_tiles = [None] * QB

            def load_q(qb):
                t = qt_pool.tile([128, 4, 512], BF16, tag="qt")
                nc.sync.dma_start(
                    t[:, :, :],
                    qt_d.rearrange("(c p) s -> p c s", p=128)[
                        :, :, qb * 512 : (qb + 1) * 512
                    ],
                )
                qt_tiles[qb] = t

            load_q(0)
            load_q(1)

            # ---- consts via SWDGE (gpsimd) so they don't block the HWDGE queue
            ident = consts.tile([128, 128], F32)
            make_identity(nc, ident[:, :])
            wq = consts.tile([128, 4, DK], BF16)
            wk = consts.tile([128, 4, DK], BF16)
            wv = consts.tile([128, 4, DK + 1], BF16)
            wvl = consts.tile([1, DK + 1], BF16)
            nc.gpsimd.dma_start(wq[:, :, :], wq_d.rearrange("(c p) k -> p c k", p=128))
            nc.gpsimd.dma_start(wk[:, :, :], wk_d.rearrange("(c p) k -> p c k", p=128))
            nc.gpsimd.dma_start(wv[:, :, :], wv_d.rearrange("(c p) k -> p c k", p=128))
            nc.gpsimd.dma_start(wvl[:, :], wvl_d)
            bq = consts.tile([128, 1], F32)
            bk = consts.tile([128, 1], F32)
            nc.gpsimd.dma_start(bq[:, :], bq_d)
            nc.gpsimd.dma_start(bk[:, :], bk_d)
            vones = consts.tile([1, SK], BF16)
            nc.gpsimd.dma_start(vones[:, :], vones_d)

            # ---- persistent K/V state ----
            khT2 = persist.tile([128, NPAIR, 128], BF16)  # pair layout, halves
            vh = persist.tile([128, SKC, DK + 1], BF16)  # natural [key, 65]

            # ---- Phase A: K path (pair layout via col-tiled projection) ----
            for bi, (r0, nr) in enumerate(kv_blocks):
                kt = kt_tiles[bi]
                npc = nr // 128
                pr0 = r0 // 256  # first pair of this block
                # ps_k dims: [partition, local pair j, half h, cols]
                ps_k = ppa.tile([128, 2, 2, 128], F32, tag="pa")
                for lc in range(npc):
                    j, h = lc // 2, lc % 2
                    dst = ps_k[h * 64 : (h + 1) * 64, j, h, :]
                    for c in range(4):
                        nc.tensor.matmul(
                            dst,
                            wk[:, c, :],
                            kt[:, c, lc * 128 : (lc + 1) * 128],
                            start=(c == 0),
                            stop=(c == 3),
                            tile_position=(0, h * 64),
                        )
                nja = (npc + 1) // 2
                njb = npc // 2
                nc.vector.tensor_scalar_add(
                    khT2[0:64, pr0 : pr0 + nja, :],
                    ps_k[0:64, 0:nja, 0, :],
                    bk[0:64, :],
                )
                if njb:
                    nc.vector.tensor_scalar_add(
                        khT2[64:128, pr0 : pr0 + njb, :],
                        ps_k[64:128, 0:njb, 1, :],
                        bk[64:128, :],
                    )

            # ---- Phase A: V path (project augmented, flip to natural) ----
            for bi, (r0, nr) in enumerate(kv_blocks):
                vt = vt_tiles[bi]
                npc = nr // 128
                kc0 = r0 // 128
                ps_v = ppa.tile([128, 512], F32, tag="pa")
                for c in range(4):
                    nc.tensor.matmul(
                        ps_v[0 : DK + 1, 0:nr],
                        wv[:, c, :],
                        vt[:, c, 0:nr],
                        start=(c == 0),
                        stop=False,
                    )
                nc.tensor.matmul(
                    ps_v[0 : DK + 1, 0:nr],
                    wvl[0:1, :],
                    vones[0:1, r0 : r0 + nr],
                    start=False,
                    stop=True,
                )
                vs = stg_pool.tile([DK + 1, 512], F32, tag="vs")
                nc.vector.tensor_copy(vs[:, 0:nr], ps_v[0 : DK + 1, 0:nr])
                pf = ppa.tile([128, 4, 128], F32, tag="pa")
                for t in range(npc):
                    nc.tensor.transpose(
                        pf[:, t, 0 : DK + 1],
                        vs[:, t * 128 : (t + 1) * 128],
                        ident[0 : DK + 1, 0 : DK + 1],
                    )
                nc.vector.tensor_copy(
                    vh[:, kc0 : kc0 + npc, :], pf[:, 0:npc, 0 : DK + 1]
                )

            # ---- Phase B: Q projection (duplicated halves) + attention ----
            def proj_q(qb):
                qt = qt_tiles[qb]
                ps_q = ppa.tile([128, 512], F32, tag="pa")
                for c in range(4):
                    nc.tensor.matmul(
                        ps_q[0:64, :],
                        wq[:, c, :],
                        qt[:, c, :],
                        start=(c == 0),
                        stop=(c == 3),
                        tile_position=(0, 0),
                    )
                    nc.tensor.matmul(
                        ps_q[64:128, :],
                        wq[:, c, :],
                        qt[:, c, :],
                        start=(c == 0),
                        stop=(c == 3),
                        tile_position=(0, 64),
                    )
                qh2 = qh_pool.tile([128, 512], BF16, tag="qh")
                nc.vector.tensor_scalar_add(qh2[:, :], ps_q[:, :], bq[:, :])
                return qh2

            qh_cur = proj_q(0)
            for qb in range(QB):
                if qb + 2 < QB:
                    load_q(qb + 2)
                qh_next = proj_q(qb + 1) if qb + 1 < QB else None

                po = ppo.tile([DK + 1, 512], F32, tag="po")
                prev = None
                for j in range(NPAIR):
                    both = 2 * j + 1 < SKC
                    width = 1024 if both else 512
                    ps_s = pps.tile([128, 1024], F32, tag="ss")
                    nc.tensor.matmul(
                        ps_s[:, 0:512],
                        khT2[0:64, j, :],
                        qh_cur[0:64, :],
                        start=True,
                        stop=True,
                        tile_position=(0, 0),
                    )
                    if both:
                        nc.tensor.matmul(
                            ps_s[:, 512:1024],
                            khT2[64:128, j, :],
                            qh_cur[64:128, :],
                            start=True,
                            stop=True,
                            tile_position=(64, 0),
                        )
                    et = et_pool.tile([128, 1024], BF16, tag="et")
                    nc.scalar.activation(et[:, 0:width], ps_s[:, 0:width], EXP)
                    if prev is not None:
                        pet, pj, pw = prev
                        for h in range(pw // 512):
                            kc = 2 * pj + h
                            nc.tensor.matmul(
                                po[:, :],
                                vh[:, kc, :],
                                pet[:, h * 512 : (h + 1) * 512],
                                start=(kc == 0),
                                stop=(kc == SKC - 1),
                            )
                    prev = (et, j, width)
                pet, pj, pw = prev
                for h in range(pw // 512):
                    kc = 2 * pj + h
                    nc.tensor.matmul(
                        po[:, :],
                        vh[:, kc, :],
                        pet[:, h * 512 : (h + 1) * 512],
                        start=(kc == 0),
                        stop=(kc == SKC - 1),
                    )

                # finalize: flip to natural [q, v], scale by 1/rowsum
                ot = stg_pool.tile([DK + 1, 512], F32, tag="ot")
                nc.vector.tensor_copy(ot[:, :], po[:, :])
                pf = ppa.tile([128, 4, 128], F32, tag="pa")
                for t in range(4):
                    nc.tensor.transpose(
                        pf[:, t, 0 : DK + 1],
                        ot[:, t * 128 : (t + 1) * 128],
                        ident[0 : DK + 1, 0 : DK + 1],
                    )
                rec = rec_pool.tile([128, 4, 1], F32, tag="r")
                nc.vector.reciprocal(rec[:, :, :], pf[:, :, DK : DK + 1])
                ostage = out_pool.tile([128, 4, DK], F32, tag="os")
                for t in range(4):
                    nc.vector.tensor_scalar_mul(
                        ostage[:, t, :], pf[:, t, 0:DK], rec[:, t, :]
                    )
                nc.sync.dma_start(
                    out_d[qb * 512 : (qb + 1) * 512, :].rearrange(
                        "(t p) v -> p t v", p=128
                    ),
                    ostage[:, :, :],
                )
                qh_cur = qh_next

    nc.compile()
    return nc


_NC_CACHE: dict = {}


def prepare(inputs):
    """Host-side preprocessing: returns (nc, in_maps)."""
    q = np.asarray(inputs["q"], dtype=np.float32)
    k = np.asarray(inputs["k"], dtype=np.float32)
    v = np.asarray(inputs["v"], dtype=np.float32)
    mask = np.asarray(inputs["mask"])
    Wq = np.asarray(inputs["Wq"], dtype=np.float32)
    bq = np.asarray(inputs["bq"], dtype=np.float32)
    Wk = np.asarray(inputs["Wk"], dtype=np.float32)
    bk = np.asarray(inputs["bk"], dtype=np.float32)
    Wv = np.asarray(inputs["Wv"], dtype=np.float32)
    bv = np.asarray(inputs["bv"], dtype=np.float32)
    B = q.shape[0]
    assert q.shape == (B, S, D) and B == N_CORES

    idxs = [np.nonzero(mask[b])[0] for b in range(B)]
    max_cnt = max(len(ix) for ix in idxs)
    SK = ((max_cnt + 127) // 128) * 128
    SK = max(SK, 512)

    scale = np.float32(1.0 / np.sqrt(np.float32(DK)))
    wq8 = (Wq * scale).astype(NP_BF16)
    wk8 = Wk.astype(NP_BF16)
    wv_aug = np.concatenate([Wv, np.zeros((D, 1), np.float32)], axis=1).astype(NP_BF16)
    wvl = np.concatenate([bv, np.ones(1, np.float32)]).reshape(1, DK + 1).astype(
        NP_BF16
    )
    bq2 = np.concatenate([bq * scale, bq * scale]).reshape(128, 1).astype(np.float32)
    bk2 = np.concatenate([bk, bk]).reshape(128, 1).astype(np.float32)

    in_maps = []
    for b in range(B):
        ix = idxs[b]
        cnt = len(ix)
        kt = np.zeros((D, SK), NP_BF16)
        vt = np.zeros((D, SK), NP_BF16)
        kt[:, :cnt] = k[b][ix].astype(NP_BF16).T
        vt[:, :cnt] = v[b][ix].astype(NP_BF16).T
        vones = np.zeros((1, SK), NP_BF16)
        vones[0, :cnt] = 1.0
        in_maps.append(
            dict(
                qt=np.ascontiguousarray(q[b].astype(NP_BF16).T),
                kt=np.ascontiguousarray(kt),
                vt=np.ascontiguousarray(vt),
                vones=vones,
                wq=wq8,
                wk=wk8,
                wv=wv_aug,
                wvl=wvl,
                bq2=bq2,
                bk2=bk2,
            )
        )

    if SK not in _NC_CACHE:
        _NC_CACHE[SK] = _build_nc(SK)
    return _NC_CACHE[SK], in_maps


def kernel(**inputs) -> np.ndarray:
    nc, in_maps = prepare(inputs)
    res = run_bass_kernel_spmd(nc, in_maps, list(range(N_CORES)))
    out = np.stack([res.results[b]["out"] for b in range(len(in_maps))], axis=0)
    return out.astype(np.float32)
